# revision 6
# baseline (speedup 1.0000x reference)
"""Trainium2 Bass kernel for moe_routing (nn_Bool_39230231281903).

Computes, for x:[N,128], W0,W1:[128,128], b0,b1:[128]:
    route1 = mean(x, axis=1) > 0
    y0 = relu(x @ W0 + b0); y1 = relu(x @ W1 + b1)
    y = where(route1[:, None], y1, y0)

Strategy: data-parallel over 8 NeuronCores (token axis sharded, weights
replicated). Per core, tokens are processed in 128-row tiles:
  PE   : transpose x-tile (tokens,d)->(d,tokens); one float32r matmul
         against [W0|W1] (N=256).
  DVE  : routing row-sums via tensor_reduce on the natural x tile (fp32,
         sequential over d — matches the reference's mean); psum->sbuf
         copy of x^T with fp32->fp32r rounding; mask m=(sum>0).
  ACT  : t = relu(y0 * om), u = relu(y1 * m)  (per-partition scale APs —
         a nonnegative scale commutes with relu, so this applies the
         routing select for free).
  POOL : mask om=(sum<=0); y = t + u.
DMA moves 2 MiB blocks (4096 tokens) with 512B-contiguous chunks.
"""

from contextlib import ExitStack

import numpy as np

import concourse.bacc as bacc
import concourse.bass as bass
import concourse.mybir as mybir
import concourse.tile as tile
from concourse.bass_utils import run_bass_kernel_spmd

N_CORES = 8
N_TOKENS = 524288
D = 128
N_SHARD = N_TOKENS // N_CORES  # 65536

F32 = mybir.dt.float32
F32R = mybir.dt.float32r


def build_program(n_shard=N_SHARD, blk=4096, with_bias=False):
    assert n_shard % blk == 0 and blk % 128 == 0
    n_blocks = n_shard // blk
    k_tiles = blk // 128
    Relu = mybir.ActivationFunctionType.Relu

    nc = bacc.Bacc("TRN2", target_bir_lowering=False, debug=False)
    x_d = nc.dram_tensor("x", (n_shard, D), F32, kind="ExternalInput").ap()
    w01_d = nc.dram_tensor("w01", (D, 2 * D), F32, kind="ExternalInput").ap()
    ident_d = nc.dram_tensor("ident", (D, D), F32, kind="ExternalInput").ap()
    if with_bias:
        b01_d = nc.dram_tensor("b01", (1, 2 * D), F32, kind="ExternalInput").ap()
    y_d = nc.dram_tensor("y", (n_shard, D), F32, kind="ExternalOutput").ap()

    with tile.TileContext(nc) as tc, ExitStack() as ctx:
        const_pool = ctx.enter_context(tc.tile_pool(name="const", bufs=1))
        xin_pool = ctx.enter_context(tc.tile_pool(name="xin", bufs=2))
        yout_pool = ctx.enter_context(tc.tile_pool(name="yout", bufs=2))
        xt_pool = ctx.enter_context(tc.tile_pool(name="xt", bufs=4))
        tu_pool = ctx.enter_context(tc.tile_pool(name="tu", bufs=8))
        m_pool = ctx.enter_context(tc.tile_pool(name="m", bufs=8))
        pxt_pool = ctx.enter_context(tc.tile_pool(name="pxt", bufs=2, space="PSUM"))
        py_pool = ctx.enter_context(tc.tile_pool(name="py", bufs=2, space="PSUM"))

        w01_f32 = const_pool.tile([D, 2 * D], F32)
        nc.sync.dma_start(w01_f32[:], w01_d)
        w01_sb = const_pool.tile([D, 2 * D], F32R)
        nc.vector.tensor_copy(w01_sb[:], w01_f32[:])
        ident_sb = const_pool.tile([D, D], F32)
        nc.sync.dma_start(ident_sb[:], ident_d)
        if with_bias:
            ones_row = const_pool.tile([1, D], F32R)
            nc.vector.memset(ones_row[:], 1.0)
            b01_f32 = const_pool.tile([1, 2 * D], F32)
            nc.sync.dma_start(b01_f32[:], b01_d)
            b01_sb = const_pool.tile([1, 2 * D], F32R)
            nc.vector.tensor_copy(b01_sb[:], b01_f32[:])

        for b in range(n_blocks):
            x_blk = x_d[b * blk : (b + 1) * blk, :].rearrange(
                "(k p) d -> p k d", p=128
            )
            xin = xin_pool.tile([128, k_tiles, D], F32)
            nc.sync.dma_start(xin[:], x_blk)
            yout = yout_pool.tile([128, k_tiles, D], F32)
            for k in range(k_tiles):
                xsl = xin[:, k, :]
                s = m_pool.tile([D, 1], F32)
                nc.vector.tensor_reduce(
                    s[:], xsl, mybir.AxisListType.X, mybir.AluOpType.add
                )
                pxt = pxt_pool.tile([D, D], F32)
                nc.tensor.transpose(pxt[:], xsl, ident_sb[:])
                xt = xt_pool.tile([D, D], F32R)
                nc.vector.tensor_copy(xt[:], pxt[:])
                py = py_pool.tile([D, 2 * D], F32)
                nc.tensor.matmul(
                    py[:],
                    xt[:],
                    w01_sb[:],
                    start=True,
                    stop=not with_bias,
                )
                if with_bias:
                    nc.tensor.matmul(
                        py[:],
                        ones_row[:],
                        b01_sb[:],
                        start=False,
                        stop=True,
                    )
                m = m_pool.tile([D, 1], F32)
                om = m_pool.tile([D, 1], F32)
                nc.vector.tensor_scalar(
                    m[:], s[:], 0.0, None, mybir.AluOpType.is_gt
                )
                nc.gpsimd.tensor_scalar(
                    om[:], s[:], 0.0, None, mybir.AluOpType.is_le
                )
                t = tu_pool.tile([D, D], F32)
                u = tu_pool.tile([D, D], F32)
                nc.scalar.activation(t[:], py[:, 0:D], Relu, scale=om[:])
                nc.scalar.activation(u[:], py[:, D : 2 * D], Relu, scale=m[:])
                nc.gpsimd.tensor_add(yout[:, k, :], t[:], u[:])
            y_blk = y_d[b * blk : (b + 1) * blk, :].rearrange(
                "(k p) d -> p k d", p=128
            )
            nc.sync.dma_start(y_blk, yout[:])

    nc.compile()
    return nc


def make_in_maps(x, W0, b0, W1, b1, n_cores=N_CORES, with_bias=False):
    n_shard = x.shape[0] // n_cores
    w01 = np.ascontiguousarray(np.concatenate([W0, W1], axis=1), dtype=np.float32)
    ident = np.eye(D, dtype=np.float32)
    in_maps = []
    for c in range(n_cores):
        im = {
            "x": np.ascontiguousarray(x[c * n_shard : (c + 1) * n_shard]),
            "w01": w01,
            "ident": ident,
        }
        if with_bias:
            im["b01"] = np.concatenate([b0, b1]).reshape(1, 2 * D).astype(np.float32)
        in_maps.append(im)
    return in_maps


def kernel(x, W0, b0, W1, b1):
    x = np.asarray(x, dtype=np.float32)
    W0 = np.asarray(W0, dtype=np.float32)
    W1 = np.asarray(W1, dtype=np.float32)
    b0 = np.asarray(b0, dtype=np.float32)
    b1 = np.asarray(b1, dtype=np.float32)
    with_bias = bool(np.any(b0) or np.any(b1))

    nc = build_program(with_bias=with_bias)
    in_maps = make_in_maps(x, W0, b0, W1, b1, with_bias=with_bias)
    res = run_bass_kernel_spmd(nc, in_maps, core_ids=list(range(N_CORES)))
    return np.concatenate([r["y"] for r in res.results], axis=0)


# revision 13
# speedup vs baseline: 2.0758x; 2.0758x over previous
"""Trainium2 Bass kernel for moe_routing (nn_Bool_39230231281903).

Computes, for x:[N,128], W0,W1:[128,128], b0,b1:[128]:
    route1 = mean(x, axis=1) > 0
    y0 = relu(x @ W0 + b0); y1 = relu(x @ W1 + b1)
    y = where(route1[:, None], y1, y0)

Strategy: data-parallel over 8 NeuronCores (token axis sharded, weights
replicated). The device works entirely in transposed (feature-major)
space so every HBM transfer is long-contiguous per partition:

  host  : x is sharded and transposed per core to xT:[128, 65536];
          routing masks are precomputed with a strictly-sequential fp32
          row-sum (bit-identical to the reference's jnp.mean on this
          backend) and shipped as a [128, 512] tile, one row per
          512-token group.
  PE    : per 512-token group, two float32r matmuls with W0 / W1 as the
          stationary operand and xT streaming: y0T = W0^T @ xT,
          y1T = W1^T @ xT (psum, token on the free axis).
  DVE   : copy_predicated(y0T <- y1T where mask) — the routing select,
          with the group's mask row partition-broadcast across all 128
          output features.
  ACT   : relu eviction psum -> sbuf yT block.
  DMA   : 4 MiB blocks in/out, 32 KiB contiguous per partition.

Output returns transposed per core; the host transposes back and
concatenates.
"""

from contextlib import ExitStack

import numpy as np

import concourse.bacc as bacc
import concourse.bass as bass
import concourse.mybir as mybir
import concourse.tile as tile
from concourse.bass_utils import run_bass_kernel_spmd

N_CORES = 8
N_TOKENS = 524288
D = 128
N_SHARD = N_TOKENS // N_CORES  # 65536
GRP = 512  # tokens per psum group (one matmul free-dim)

F32 = mybir.dt.float32
F32R = mybir.dt.float32r


def build_program(n_shard=N_SHARD, blk=4096, with_bias=False):
    assert n_shard % blk == 0 and blk % GRP == 0
    n_blocks = n_shard // blk
    n_groups = blk // GRP
    total_groups = n_shard // GRP
    Relu = mybir.ActivationFunctionType.Relu

    nc = bacc.Bacc("TRN2", target_bir_lowering=False, debug=False)
    xt_d = nc.dram_tensor("xt", (D, n_shard), F32R, kind="ExternalInput").ap()
    w0_d = nc.dram_tensor("w0", (D, D), F32R, kind="ExternalInput").ap()
    w1_d = nc.dram_tensor("w1", (D, D), F32R, kind="ExternalInput").ap()
    msk_d = nc.dram_tensor(
        "msk", (1, n_shard), mybir.dt.uint8, kind="ExternalInput"
    ).ap()
    if with_bias:
        b01_d = nc.dram_tensor("b01", (1, 2 * D), F32, kind="ExternalInput").ap()
    yt_d = nc.dram_tensor("yt", (D, n_shard), F32, kind="ExternalOutput").ap()

    with tile.TileContext(nc) as tc, ExitStack() as ctx:
        const_pool = ctx.enter_context(tc.tile_pool(name="const", bufs=1))
        xin_pool = ctx.enter_context(tc.tile_pool(name="xin", bufs=2))
        yout_pool = ctx.enter_context(tc.tile_pool(name="yout", bufs=2))
        py0_pool = ctx.enter_context(tc.tile_pool(name="py0", bufs=3, space="PSUM"))
        py1_pool = ctx.enter_context(tc.tile_pool(name="py1", bufs=3, space="PSUM"))
        msk_pool = ctx.enter_context(tc.tile_pool(name="mskp", bufs=2))
        mb_pool = ctx.enter_context(tc.tile_pool(name="mb", bufs=4))

        w0_sb = const_pool.tile([D, D], F32R)
        nc.sync.dma_start(w0_sb[:], w0_d)
        w1_sb = const_pool.tile([D, D], F32R)
        nc.sync.dma_start(w1_sb[:], w1_d)
        if with_bias:
            ones_row = const_pool.tile([1, GRP], F32R)
            nc.vector.memset(ones_row[:], 1.0)
            b01_f32 = const_pool.tile([1, 2 * D], F32)
            nc.sync.dma_start(b01_f32[:], b01_d)
            b01_sb = const_pool.tile([1, 2 * D], F32R)
            nc.vector.tensor_copy(b01_sb[:], b01_f32[:])

        CH = 2  # groups per stationary-weight chunk
        assert n_groups % CH == 0
        for b in range(n_blocks):
            xin = xin_pool.tile([D, blk], F32R)
            nc.sync.dma_start(xin[:], xt_d[:, b * blk : (b + 1) * blk])
            msk_blk = msk_pool.tile([1, blk], mybir.dt.uint8)
            nc.sync.dma_start(msk_blk[:], msk_d[:, b * blk : (b + 1) * blk])
            yout = yout_pool.tile([D, blk], F32)
            for c in range(n_groups // CH):
                gs = [c * CH + i for i in range(CH)]
                xg = [xin[:, g * GRP : (g + 1) * GRP] for g in gs]
                mb = [
                    mb_pool.tile([D, GRP], mybir.dt.uint8, name="mb") for _ in gs
                ]
                py0 = [py0_pool.tile([D, GRP], F32, name="py0") for _ in gs]
                py1 = [py1_pool.tile([D, GRP], F32, name="py1") for _ in gs]
                # group matmuls by stationary operand to minimize LDWEIGHTS
                for i, g in enumerate(gs):
                    nc.gpsimd.partition_broadcast(
                        mb[i][:], msk_blk[:, g * GRP : (g + 1) * GRP]
                    )
                for i in range(CH):
                    nc.tensor.matmul(
                        py0[i][:], w0_sb[:], xg[i], start=True, stop=not with_bias
                    )
                for i in range(CH):
                    nc.tensor.matmul(
                        py1[i][:], w1_sb[:], xg[i], start=True, stop=not with_bias
                    )
                if with_bias:
                    for i in range(CH):
                        nc.tensor.matmul(
                            py0[i][:],
                            b01_sb[:, 0:D],
                            ones_row[:],
                            start=False,
                            stop=True,
                        )
                    for i in range(CH):
                        nc.tensor.matmul(
                            py1[i][:],
                            b01_sb[:, D : 2 * D],
                            ones_row[:],
                            start=False,
                            stop=True,
                        )
                for i, g in enumerate(gs):
                    nc.vector.copy_predicated(py0[i][:], mb[i][:], py1[i][:])
                    nc.scalar.activation(
                        yout[:, g * GRP : (g + 1) * GRP], py0[i][:], Relu
                    )
            nc.sync.dma_start(yt_d[:, b * blk : (b + 1) * blk], yout[:])

    nc.compile()
    return nc


def routing_mask(x):
    """route1 = mean(x,axis=1) > 0, with a strictly-sequential fp32 sum —
    matches XLA's lowering of jnp.mean on this backend bit-exactly."""
    acc = x[:, 0].astype(np.float32).copy()
    for j in range(1, x.shape[1]):
        acc += x[:, j]
    return (acc > 0.0).astype(np.float32)


def make_in_maps(x, W0, b0, W1, b1, n_cores=N_CORES, with_bias=False):
    n_shard = x.shape[0] // n_cores
    mask = routing_mask(x)
    in_maps = []
    for c in range(n_cores):
        xs = x[c * n_shard : (c + 1) * n_shard]
        im = {
            "xt": np.ascontiguousarray(xs.T),
            "w0": np.ascontiguousarray(W0, dtype=np.float32),
            "w1": np.ascontiguousarray(W1, dtype=np.float32),
            "msk": np.ascontiguousarray(
                mask[c * n_shard : (c + 1) * n_shard]
                .reshape(1, -1)
                .astype(np.uint8)
            ),
        }
        if with_bias:
            im["b01"] = (
                np.concatenate([b0, b1]).reshape(1, 2 * D).astype(np.float32)
            )
        in_maps.append(im)
    return in_maps


def kernel(x, W0, b0, W1, b1):
    x = np.asarray(x, dtype=np.float32)
    W0 = np.asarray(W0, dtype=np.float32)
    W1 = np.asarray(W1, dtype=np.float32)
    b0 = np.asarray(b0, dtype=np.float32)
    b1 = np.asarray(b1, dtype=np.float32)
    with_bias = bool(np.any(b0) or np.any(b1))

    nc = build_program(with_bias=with_bias)
    in_maps = make_in_maps(x, W0, b0, W1, b1, with_bias=with_bias)
    res = run_bass_kernel_spmd(nc, in_maps, core_ids=list(range(N_CORES)))
    return np.concatenate([r["yt"].T for r in res.results], axis=0)


# revision 14
# speedup vs baseline: 2.2967x; 1.1064x over previous
"""Trainium2 Bass kernel for moe_routing (nn_Bool_39230231281903).

Computes, for x:[N,128], W0,W1:[128,128], b0,b1:[128]:
    route1 = mean(x, axis=1) > 0
    y0 = relu(x @ W0 + b0); y1 = relu(x @ W1 + b1)
    y = where(route1[:, None], y1, y0)

Strategy: data-parallel over 8 NeuronCores (token axis sharded, weights
replicated). The device works entirely in transposed (feature-major)
space so every HBM transfer is long-contiguous per partition:

  host  : x is sharded and transposed per core to xT:[128, 65536];
          routing masks are precomputed with a strictly-sequential fp32
          row-sum (bit-identical to the reference's jnp.mean on this
          backend) and shipped as a [128, 512] tile, one row per
          512-token group.
  PE    : per 512-token group, two float32r matmuls with W0 / W1 as the
          stationary operand and xT streaming: y0T = W0^T @ xT,
          y1T = W1^T @ xT (psum, token on the free axis).
  DVE   : copy_predicated(y0T <- y1T where mask) — the routing select,
          with the group's mask row partition-broadcast across all 128
          output features.
  ACT   : relu eviction psum -> sbuf yT block.
  DMA   : 4 MiB blocks in/out, 32 KiB contiguous per partition.

Output returns transposed per core; the host transposes back and
concatenates.
"""

from contextlib import ExitStack

import numpy as np

import concourse.bacc as bacc
import concourse.bass as bass
import concourse.mybir as mybir
import concourse.tile as tile
from concourse.bass_utils import run_bass_kernel_spmd

N_CORES = 8
N_TOKENS = 524288
D = 128
N_SHARD = N_TOKENS // N_CORES  # 65536
GRP = 512  # tokens per psum group (one matmul free-dim)

F32 = mybir.dt.float32
F32R = mybir.dt.float32r


def build_program(n_shard=N_SHARD, blk=2048, with_bias=False):
    assert n_shard % blk == 0 and blk % GRP == 0
    n_blocks = n_shard // blk
    n_groups = blk // GRP
    total_groups = n_shard // GRP
    Relu = mybir.ActivationFunctionType.Relu

    nc = bacc.Bacc("TRN2", target_bir_lowering=False, debug=False)
    xt_d = nc.dram_tensor("xt", (D, n_shard), F32R, kind="ExternalInput").ap()
    w0_d = nc.dram_tensor("w0", (D, D), F32R, kind="ExternalInput").ap()
    w1_d = nc.dram_tensor("w1", (D, D), F32R, kind="ExternalInput").ap()
    msk_d = nc.dram_tensor(
        "msk", (1, n_shard), mybir.dt.uint8, kind="ExternalInput"
    ).ap()
    if with_bias:
        b01_d = nc.dram_tensor("b01", (1, 2 * D), F32, kind="ExternalInput").ap()
    yt_d = nc.dram_tensor("yt", (D, n_shard), F32, kind="ExternalOutput").ap()

    with tile.TileContext(nc) as tc, ExitStack() as ctx:
        const_pool = ctx.enter_context(tc.tile_pool(name="const", bufs=1))
        xin_pool = ctx.enter_context(tc.tile_pool(name="xin", bufs=3))
        yout_pool = ctx.enter_context(tc.tile_pool(name="yout", bufs=3))
        py0_pool = ctx.enter_context(tc.tile_pool(name="py0", bufs=3, space="PSUM"))
        py1_pool = ctx.enter_context(tc.tile_pool(name="py1", bufs=3, space="PSUM"))
        mb_pool = ctx.enter_context(tc.tile_pool(name="mb", bufs=3))

        msk_sb = const_pool.tile([1, n_shard], mybir.dt.uint8)
        nc.sync.dma_start(msk_sb[:], msk_d)
        w0_sb = const_pool.tile([D, D], F32R)
        nc.sync.dma_start(w0_sb[:], w0_d)
        w1_sb = const_pool.tile([D, D], F32R)
        nc.sync.dma_start(w1_sb[:], w1_d)
        if with_bias:
            ones_row = const_pool.tile([1, GRP], F32R)
            nc.vector.memset(ones_row[:], 1.0)
            b01_f32 = const_pool.tile([1, 2 * D], F32)
            nc.sync.dma_start(b01_f32[:], b01_d)
            b01_sb = const_pool.tile([1, 2 * D], F32R)
            nc.vector.tensor_copy(b01_sb[:], b01_f32[:])

        CH = 2  # groups per stationary-weight chunk
        assert n_groups % CH == 0
        for b in range(n_blocks):
            xin = xin_pool.tile([D, blk], F32R)
            nc.sync.dma_start(xin[:], xt_d[:, b * blk : (b + 1) * blk])
            yout = yout_pool.tile([D, blk], F32)
            for c in range(n_groups // CH):
                gs = [c * CH + i for i in range(CH)]
                xg = [xin[:, g * GRP : (g + 1) * GRP] for g in gs]
                mb = mb_pool.tile([D, CH * GRP], mybir.dt.uint8, name="mb")
                py0 = [py0_pool.tile([D, GRP], F32, name="py0") for _ in gs]
                py1 = [py1_pool.tile([D, GRP], F32, name="py1") for _ in gs]
                goff = b * blk + gs[0] * GRP
                nc.gpsimd.partition_broadcast(
                    mb[:], msk_sb[:, goff : goff + CH * GRP]
                )
                # group matmuls by stationary operand to minimize LDWEIGHTS
                for i in range(CH):
                    nc.tensor.matmul(
                        py0[i][:], w0_sb[:], xg[i], start=True, stop=not with_bias
                    )
                for i in range(CH):
                    nc.tensor.matmul(
                        py1[i][:], w1_sb[:], xg[i], start=True, stop=not with_bias
                    )
                if with_bias:
                    for i in range(CH):
                        nc.tensor.matmul(
                            py0[i][:],
                            b01_sb[:, 0:D],
                            ones_row[:],
                            start=False,
                            stop=True,
                        )
                    for i in range(CH):
                        nc.tensor.matmul(
                            py1[i][:],
                            b01_sb[:, D : 2 * D],
                            ones_row[:],
                            start=False,
                            stop=True,
                        )
                for i, g in enumerate(gs):
                    nc.vector.copy_predicated(
                        py0[i][:], mb[:, i * GRP : (i + 1) * GRP], py1[i][:]
                    )
                    nc.scalar.activation(
                        yout[:, g * GRP : (g + 1) * GRP], py0[i][:], Relu
                    )
            nc.sync.dma_start(yt_d[:, b * blk : (b + 1) * blk], yout[:])

    nc.compile()
    return nc


def routing_mask(x):
    """route1 = mean(x,axis=1) > 0, with a strictly-sequential fp32 sum —
    matches XLA's lowering of jnp.mean on this backend bit-exactly."""
    acc = x[:, 0].astype(np.float32).copy()
    for j in range(1, x.shape[1]):
        acc += x[:, j]
    return (acc > 0.0).astype(np.float32)


def make_in_maps(x, W0, b0, W1, b1, n_cores=N_CORES, with_bias=False):
    n_shard = x.shape[0] // n_cores
    mask = routing_mask(x)
    in_maps = []
    for c in range(n_cores):
        xs = x[c * n_shard : (c + 1) * n_shard]
        im = {
            "xt": np.ascontiguousarray(xs.T),
            "w0": np.ascontiguousarray(W0, dtype=np.float32),
            "w1": np.ascontiguousarray(W1, dtype=np.float32),
            "msk": np.ascontiguousarray(
                mask[c * n_shard : (c + 1) * n_shard]
                .reshape(1, -1)
                .astype(np.uint8)
            ),
        }
        if with_bias:
            im["b01"] = (
                np.concatenate([b0, b1]).reshape(1, 2 * D).astype(np.float32)
            )
        in_maps.append(im)
    return in_maps


def kernel(x, W0, b0, W1, b1):
    x = np.asarray(x, dtype=np.float32)
    W0 = np.asarray(W0, dtype=np.float32)
    W1 = np.asarray(W1, dtype=np.float32)
    b0 = np.asarray(b0, dtype=np.float32)
    b1 = np.asarray(b1, dtype=np.float32)
    with_bias = bool(np.any(b0) or np.any(b1))

    nc = build_program(with_bias=with_bias)
    in_maps = make_in_maps(x, W0, b0, W1, b1, with_bias=with_bias)
    res = run_bass_kernel_spmd(nc, in_maps, core_ids=list(range(N_CORES)))
    return np.concatenate([r["yt"].T for r in res.results], axis=0)


# revision 17
# speedup vs baseline: 2.4392x; 1.0620x over previous
"""Trainium2 Bass kernel for moe_routing (nn_Bool_39230231281903).

Computes, for x:[N,128], W0,W1:[128,128], b0,b1:[128]:
    route1 = mean(x, axis=1) > 0
    y0 = relu(x @ W0 + b0); y1 = relu(x @ W1 + b1)
    y = where(route1[:, None], y1, y0)

Strategy: data-parallel over 8 NeuronCores (token axis sharded, weights
replicated). The device works entirely in transposed (feature-major)
space so every HBM transfer is long-contiguous per partition:

  host  : x is sharded and transposed per core to xT:[128, 65536];
          routing masks are precomputed with a strictly-sequential fp32
          row-sum (bit-identical to the reference's jnp.mean on this
          backend) and shipped as a [128, 512] tile, one row per
          512-token group.
  PE    : per 512-token group, two float32r matmuls with W0 / W1 as the
          stationary operand and xT streaming: y0T = W0^T @ xT,
          y1T = W1^T @ xT (psum, token on the free axis).
  DVE   : copy_predicated(y0T <- y1T where mask) — the routing select,
          with the group's mask row partition-broadcast across all 128
          output features.
  ACT   : relu eviction psum -> sbuf yT block.
  DMA   : 4 MiB blocks in/out, 32 KiB contiguous per partition.

Output returns transposed per core; the host transposes back and
concatenates.
"""

from contextlib import ExitStack

import numpy as np

import concourse.bacc as bacc
import concourse.bass as bass
import concourse.mybir as mybir
import concourse.tile as tile
from concourse.bass_utils import run_bass_kernel_spmd

N_CORES = 8
N_TOKENS = 524288
D = 128
N_SHARD = N_TOKENS // N_CORES  # 65536
GRP = 512  # tokens per psum group (one matmul free-dim)

F32 = mybir.dt.float32
F32R = mybir.dt.float32r


def build_program(n_shard=N_SHARD, blk=2048, with_bias=False):
    assert n_shard % blk == 0 and blk % GRP == 0
    n_blocks = n_shard // blk
    n_groups = blk // GRP
    total_groups = n_shard // GRP
    Relu = mybir.ActivationFunctionType.Relu

    nc = bacc.Bacc("TRN2", target_bir_lowering=False, debug=False)
    xt_d = nc.dram_tensor("xt", (D, n_shard), F32R, kind="ExternalInput").ap()
    w0_d = nc.dram_tensor("w0", (D, D), F32R, kind="ExternalInput").ap()
    w1_d = nc.dram_tensor("w1", (D, D), F32R, kind="ExternalInput").ap()
    msk_d = nc.dram_tensor(
        "msk", (1, n_shard), mybir.dt.uint8, kind="ExternalInput"
    ).ap()
    if with_bias:
        b01_d = nc.dram_tensor("b01", (1, 2 * D), F32, kind="ExternalInput").ap()
    yt_d = nc.dram_tensor("yt", (D, n_shard), F32, kind="ExternalOutput").ap()

    with tile.TileContext(nc) as tc, ExitStack() as ctx:
        const_pool = ctx.enter_context(tc.tile_pool(name="const", bufs=1))
        xin_pool = ctx.enter_context(tc.tile_pool(name="xin", bufs=3))
        yout_pool = ctx.enter_context(tc.tile_pool(name="yout", bufs=3))
        py0_pool = ctx.enter_context(tc.tile_pool(name="py0", bufs=3, space="PSUM"))
        py1_pool = ctx.enter_context(tc.tile_pool(name="py1", bufs=3, space="PSUM"))
        mb_pool = ctx.enter_context(tc.tile_pool(name="mb", bufs=3))

        # block-0 input first so the big stream starts immediately
        xin0 = xin_pool.tile([D, blk], F32R, name="xin", tag="xin")
        nc.sync.dma_start(xin0[:], xt_d[:, 0:blk])
        w0_sb = const_pool.tile([D, D], F32R)
        nc.sync.dma_start(w0_sb[:], w0_d)
        w1_sb = const_pool.tile([D, D], F32R)
        nc.sync.dma_start(w1_sb[:], w1_d)
        msk_sb = const_pool.tile([1, n_shard], mybir.dt.uint8)
        nc.sync.dma_start(msk_sb[:], msk_d)
        if with_bias:
            ones_row = const_pool.tile([1, GRP], F32R)
            nc.vector.memset(ones_row[:], 1.0)
            b01_f32 = const_pool.tile([1, 2 * D], F32)
            nc.sync.dma_start(b01_f32[:], b01_d)
            b01_sb = const_pool.tile([1, 2 * D], F32R)
            nc.vector.tensor_copy(b01_sb[:], b01_f32[:])

        CH = 2  # groups per stationary-weight chunk
        assert n_groups % CH == 0
        for b in range(n_blocks):
            if b == 0:
                xin = xin0
            else:
                xin = xin_pool.tile([D, blk], F32R, name="xin", tag="xin")
                nc.sync.dma_start(xin[:], xt_d[:, b * blk : (b + 1) * blk])
            yout = yout_pool.tile([D, blk], F32)
            for c in range(n_groups // CH):
                gs = [c * CH + i for i in range(CH)]
                xg = [xin[:, g * GRP : (g + 1) * GRP] for g in gs]
                mb = mb_pool.tile([D, CH * GRP], mybir.dt.uint8, name="mb")
                py0 = [py0_pool.tile([D, GRP], F32, name="py0") for _ in gs]
                py1 = [py1_pool.tile([D, GRP], F32, name="py1") for _ in gs]
                goff = b * blk + gs[0] * GRP
                nc.gpsimd.partition_broadcast(
                    mb[:], msk_sb[:, goff : goff + CH * GRP]
                )
                # group matmuls by stationary operand to minimize LDWEIGHTS
                for i in range(CH):
                    nc.tensor.matmul(
                        py0[i][:], w0_sb[:], xg[i], start=True, stop=not with_bias
                    )
                for i in range(CH):
                    nc.tensor.matmul(
                        py1[i][:], w1_sb[:], xg[i], start=True, stop=not with_bias
                    )
                if with_bias:
                    for i in range(CH):
                        nc.tensor.matmul(
                            py0[i][:],
                            b01_sb[:, 0:D],
                            ones_row[:],
                            start=False,
                            stop=True,
                        )
                    for i in range(CH):
                        nc.tensor.matmul(
                            py1[i][:],
                            b01_sb[:, D : 2 * D],
                            ones_row[:],
                            start=False,
                            stop=True,
                        )
                for i, g in enumerate(gs):
                    nc.vector.copy_predicated(
                        py0[i][:], mb[:, i * GRP : (i + 1) * GRP], py1[i][:]
                    )
                    nc.scalar.activation(
                        yout[:, g * GRP : (g + 1) * GRP], py0[i][:], Relu
                    )
            nc.sync.dma_start(yt_d[:, b * blk : (b + 1) * blk], yout[:])

    nc.compile()
    return nc


def routing_mask(x):
    """route1 = mean(x,axis=1) > 0, with a strictly-sequential fp32 sum —
    matches XLA's lowering of jnp.mean on this backend bit-exactly."""
    acc = x[:, 0].astype(np.float32).copy()
    for j in range(1, x.shape[1]):
        acc += x[:, j]
    return (acc > 0.0).astype(np.float32)


def make_in_maps(x, W0, b0, W1, b1, n_cores=N_CORES, with_bias=False):
    n_shard = x.shape[0] // n_cores
    mask = routing_mask(x)
    in_maps = []
    for c in range(n_cores):
        xs = x[c * n_shard : (c + 1) * n_shard]
        im = {
            "xt": np.ascontiguousarray(xs.T),
            "w0": np.ascontiguousarray(W0, dtype=np.float32),
            "w1": np.ascontiguousarray(W1, dtype=np.float32),
            "msk": np.ascontiguousarray(
                mask[c * n_shard : (c + 1) * n_shard]
                .reshape(1, -1)
                .astype(np.uint8)
            ),
        }
        if with_bias:
            im["b01"] = (
                np.concatenate([b0, b1]).reshape(1, 2 * D).astype(np.float32)
            )
        in_maps.append(im)
    return in_maps


def kernel(x, W0, b0, W1, b1):
    x = np.asarray(x, dtype=np.float32)
    W0 = np.asarray(W0, dtype=np.float32)
    W1 = np.asarray(W1, dtype=np.float32)
    b0 = np.asarray(b0, dtype=np.float32)
    b1 = np.asarray(b1, dtype=np.float32)
    with_bias = bool(np.any(b0) or np.any(b1))

    nc = build_program(with_bias=with_bias)
    in_maps = make_in_maps(x, W0, b0, W1, b1, with_bias=with_bias)
    res = run_bass_kernel_spmd(nc, in_maps, core_ids=list(range(N_CORES)))
    return np.concatenate([r["yt"].T for r in res.results], axis=0)


# revision 18
# speedup vs baseline: 2.5425x; 1.0423x over previous
"""Trainium2 Bass kernel for moe_routing (nn_Bool_39230231281903).

Computes, for x:[N,128], W0,W1:[128,128], b0,b1:[128]:
    route1 = mean(x, axis=1) > 0
    y0 = relu(x @ W0 + b0); y1 = relu(x @ W1 + b1)
    y = where(route1[:, None], y1, y0)

Strategy: data-parallel over 8 NeuronCores (token axis sharded, weights
replicated). The device works entirely in transposed (feature-major)
space so every HBM transfer is long-contiguous per partition:

  host  : x is sharded and transposed per core to xT:[128, 65536];
          routing masks are precomputed with a strictly-sequential fp32
          row-sum (bit-identical to the reference's jnp.mean on this
          backend) and shipped as a [128, 512] tile, one row per
          512-token group.
  PE    : per 512-token group, two float32r matmuls with W0 / W1 as the
          stationary operand and xT streaming: y0T = W0^T @ xT,
          y1T = W1^T @ xT (psum, token on the free axis).
  DVE   : copy_predicated(y0T <- y1T where mask) — the routing select,
          with the group's mask row partition-broadcast across all 128
          output features.
  ACT   : relu eviction psum -> sbuf yT block.
  DMA   : 4 MiB blocks in/out, 32 KiB contiguous per partition.

Output returns transposed per core; the host transposes back and
concatenates.
"""

from contextlib import ExitStack

import numpy as np

import concourse.bacc as bacc
import concourse.bass as bass
import concourse.mybir as mybir
import concourse.tile as tile
from concourse.bass_utils import run_bass_kernel_spmd

N_CORES = 8
N_TOKENS = 524288
D = 128
N_SHARD = N_TOKENS // N_CORES  # 65536
GRP = 512  # tokens per psum group (one matmul free-dim)

F32 = mybir.dt.float32
F32R = mybir.dt.float32r


def build_program(n_shard=N_SHARD, blk=2048, with_bias=False):
    assert n_shard % blk == 0 and blk % GRP == 0
    n_blocks = n_shard // blk
    n_groups = blk // GRP
    total_groups = n_shard // GRP
    Relu = mybir.ActivationFunctionType.Relu

    nc = bacc.Bacc("TRN2", target_bir_lowering=False, debug=False)
    xt_d = nc.dram_tensor("xt", (D, n_shard), F32R, kind="ExternalInput").ap()
    w0_d = nc.dram_tensor("w0", (D, D), F32R, kind="ExternalInput").ap()
    w1_d = nc.dram_tensor("w1", (D, D), F32R, kind="ExternalInput").ap()
    msk_d = nc.dram_tensor(
        "msk", (1, n_shard), mybir.dt.uint8, kind="ExternalInput"
    ).ap()
    if with_bias:
        b01_d = nc.dram_tensor("b01", (1, 2 * D), F32, kind="ExternalInput").ap()
    yt_d = nc.dram_tensor("yt", (D, n_shard), F32, kind="ExternalOutput").ap()

    with tile.TileContext(nc) as tc, ExitStack() as ctx:
        const_pool = ctx.enter_context(tc.tile_pool(name="const", bufs=1))
        xin_pool = ctx.enter_context(tc.tile_pool(name="xin", bufs=6))
        yout_pool = ctx.enter_context(tc.tile_pool(name="yout", bufs=3))
        py0_pool = ctx.enter_context(tc.tile_pool(name="py0", bufs=3, space="PSUM"))
        py1_pool = ctx.enter_context(tc.tile_pool(name="py1", bufs=3, space="PSUM"))
        mb_pool = ctx.enter_context(tc.tile_pool(name="mb", bufs=3))

        # block-0 input first so the big stream starts immediately
        xin0 = xin_pool.tile([D, blk], F32R, name="xin", tag="xin")
        nc.sync.dma_start(xin0[:], xt_d[:, 0:blk])
        w0_sb = const_pool.tile([D, D], F32R)
        nc.sync.dma_start(w0_sb[:], w0_d)
        w1_sb = const_pool.tile([D, D], F32R)
        nc.sync.dma_start(w1_sb[:], w1_d)
        msk_sb = const_pool.tile([1, n_shard], mybir.dt.uint8)
        nc.sync.dma_start(msk_sb[:], msk_d)
        if with_bias:
            ones_row = const_pool.tile([1, GRP], F32R)
            nc.vector.memset(ones_row[:], 1.0)
            b01_f32 = const_pool.tile([1, 2 * D], F32)
            nc.sync.dma_start(b01_f32[:], b01_d)
            b01_sb = const_pool.tile([1, 2 * D], F32R)
            nc.vector.tensor_copy(b01_sb[:], b01_f32[:])

        CH = 2  # groups per stationary-weight chunk
        assert n_groups % CH == 0
        for b in range(n_blocks):
            if b == 0:
                xin = xin0
            else:
                xin = xin_pool.tile([D, blk], F32R, name="xin", tag="xin")
                nc.sync.dma_start(xin[:], xt_d[:, b * blk : (b + 1) * blk])
            yout = yout_pool.tile([D, blk], F32)
            for c in range(n_groups // CH):
                gs = [c * CH + i for i in range(CH)]
                xg = [xin[:, g * GRP : (g + 1) * GRP] for g in gs]
                mb = mb_pool.tile([D, CH * GRP], mybir.dt.uint8, name="mb")
                py0 = [py0_pool.tile([D, GRP], F32, name="py0") for _ in gs]
                py1 = [py1_pool.tile([D, GRP], F32, name="py1") for _ in gs]
                goff = b * blk + gs[0] * GRP
                nc.gpsimd.partition_broadcast(
                    mb[:], msk_sb[:, goff : goff + CH * GRP]
                )
                # group matmuls by stationary operand to minimize LDWEIGHTS
                for i in range(CH):
                    nc.tensor.matmul(
                        py0[i][:], w0_sb[:], xg[i], start=True, stop=not with_bias
                    )
                for i in range(CH):
                    nc.tensor.matmul(
                        py1[i][:], w1_sb[:], xg[i], start=True, stop=not with_bias
                    )
                if with_bias:
                    for i in range(CH):
                        nc.tensor.matmul(
                            py0[i][:],
                            b01_sb[:, 0:D],
                            ones_row[:],
                            start=False,
                            stop=True,
                        )
                    for i in range(CH):
                        nc.tensor.matmul(
                            py1[i][:],
                            b01_sb[:, D : 2 * D],
                            ones_row[:],
                            start=False,
                            stop=True,
                        )
                for i, g in enumerate(gs):
                    nc.vector.copy_predicated(
                        py0[i][:], mb[:, i * GRP : (i + 1) * GRP], py1[i][:]
                    )
                    nc.scalar.activation(
                        yout[:, g * GRP : (g + 1) * GRP], py0[i][:], Relu
                    )
            nc.sync.dma_start(yt_d[:, b * blk : (b + 1) * blk], yout[:])

    nc.compile()
    return nc


def routing_mask(x):
    """route1 = mean(x,axis=1) > 0, with a strictly-sequential fp32 sum —
    matches XLA's lowering of jnp.mean on this backend bit-exactly."""
    acc = x[:, 0].astype(np.float32).copy()
    for j in range(1, x.shape[1]):
        acc += x[:, j]
    return (acc > 0.0).astype(np.float32)


def make_in_maps(x, W0, b0, W1, b1, n_cores=N_CORES, with_bias=False):
    n_shard = x.shape[0] // n_cores
    mask = routing_mask(x)
    in_maps = []
    for c in range(n_cores):
        xs = x[c * n_shard : (c + 1) * n_shard]
        im = {
            "xt": np.ascontiguousarray(xs.T),
            "w0": np.ascontiguousarray(W0, dtype=np.float32),
            "w1": np.ascontiguousarray(W1, dtype=np.float32),
            "msk": np.ascontiguousarray(
                mask[c * n_shard : (c + 1) * n_shard]
                .reshape(1, -1)
                .astype(np.uint8)
            ),
        }
        if with_bias:
            im["b01"] = (
                np.concatenate([b0, b1]).reshape(1, 2 * D).astype(np.float32)
            )
        in_maps.append(im)
    return in_maps


def kernel(x, W0, b0, W1, b1):
    x = np.asarray(x, dtype=np.float32)
    W0 = np.asarray(W0, dtype=np.float32)
    W1 = np.asarray(W1, dtype=np.float32)
    b0 = np.asarray(b0, dtype=np.float32)
    b1 = np.asarray(b1, dtype=np.float32)
    with_bias = bool(np.any(b0) or np.any(b1))

    nc = build_program(with_bias=with_bias)
    in_maps = make_in_maps(x, W0, b0, W1, b1, with_bias=with_bias)
    res = run_bass_kernel_spmd(nc, in_maps, core_ids=list(range(N_CORES)))
    return np.concatenate([r["yt"].T for r in res.results], axis=0)
